# revision 1
# baseline (speedup 1.0000x reference)
"""Trainium2 Bass kernel for nn_Decoder (DDSP-style decoder).

Pure data-parallel over batch (32 -> 4 per core x 8 cores). Per core the
oscillator rows (4 batch x 32 osc) fill the 128 partitions exactly.
Phase is synthesized in *turns* by block-affine fp32 matmuls, range-reduced
with magic-constant rounding + an accumulating -I matmul, evaluated with the
ScalarE Sin LUT (accurate on [-pi, pi]), multiplied by the matmul-synthesized
amplitude envelope, and reduced over oscillators with a selector matmul.
The noise branch runs as real-DFT basis matmuls (no FFT instructions).
"""
import numpy as np
import sys

sys.path.insert(0, "/opt/trn_rl_repo")

from concourse import bacc, mybir  # noqa: E402
from concourse.tile import TileContext  # noqa: E402
from concourse.bass_utils import run_bass_kernel_spmd  # noqa: E402

F32 = mybir.dt.float32
ALU = mybir.AluOpType
BAND_SIZES = [512, 1024, 2048, 4096, 8192, 16384]
ADJUST = {512: 0.05, 1024: 0.03, 2048: 0.05, 4096: 0.25, 8192: 1.0, 16384: 20.0}
B, C, N_OSC, NNF = 32, 64, 32, 64
NCORE = 8
BL = B // NCORE          # 4 local batch
FR = BL * NNF            # 256 noise frames per core
MAGIC = float(1.5 * 2 ** 23)
TWO_PI = float(2 * np.pi)
TOTAL = 2 * sum(BAND_SIZES)  # 64512

_nc_cache = {}

W64_ORDER = ([f'up{i}d{dd}' for i in range(3) for dd in range(3)]
             + [f'find{dd}' for dd in range(3)]
             + [w for k in range(6) for w in
                [f't{k}0', f't{k}1', f't{k}2', f't{k}3', f'bf{k}', f'nup{k}']]
             + ['ident'])
W64_IDX = {n: i for i, n in enumerate(W64_ORDER)}
B64_ORDER = ([f'up{i}' for i in range(3)] + ['fin']
             + [b for k in range(6) for b in
                [f't{k}0', f't{k}1', f't{k}2', f't{k}3', f'bf{k}', f'nup{k}']])
B64_IDX = {n: i for i, n in enumerate(B64_ORDER)}
W3264_IDX = {}
for k in range(6):
    W3264_IDX[f'amp{k}'] = 2 * k
    W3264_IDX[f'frq{k}'] = 2 * k + 1

HARM_OFF = {}
NZ_OFF = {}
_off = 0
for _k, _bs in enumerate(BAND_SIZES):
    HARM_OFF[_k] = _off
    NZ_OFF[_k] = _off + _bs
    _off += 2 * _bs


# ---------------------------------------------------------------- host consts
def _build_U(n):
    eye = np.eye(n)
    spec = np.fft.rfft(eye, axis=-1)
    spec = np.pad(spec, ((0, 0), (0, n + 1 - spec.shape[-1])))
    return np.fft.irfft(spec, n=2 * n, axis=-1) * 2  # (n, 2n)


def _interp_vecs(u):
    r = np.arange(u)
    f = (r + 0.5) / u - 0.5
    gm = np.where(r < u // 2, -f, 0.0)
    g0 = np.where(r < u // 2, 1 + f, 1 - f)
    gp = np.where(r >= u // 2, f, 0.0)
    return gm, g0, gp


BF16 = mybir.dt.bfloat16


def _slot_cfg(kk):
    # rows of one chunk's lhsT -> (slot height, slots per 128-part column)
    # base partitions limited to {0, 32, 64} by the stack
    if kk <= 64:
        return 64, 2
    return 128, 1


def _mega_layout():
    ents = [('wlin', C + 1, 4 * C), ('ubd4', BL * 4, BL * 8), ('ubd8', BL * 8, BL * 16),
            ('ubd16', BL * 16, BL * 32), ('w64', C, len(W64_ORDER) * C),
            ('w3264', C, 12 * N_OSC), ('bias64', C, len(B64_ORDER)), ('bias32', N_OSC, 12),
            ('negI', 128, 128), ('ident128', 128, 128), ('selstrip', 128, 256)]
    for k, bs in enumerate(BAND_SIZES):
        spf = bs // NNF
        nc_ = spf // 2 + 1
        ents.append((f'wc{k}', C + 1, nc_))
        if spf <= 128:
            ents.append((f'ft{k}', spf, spf))
            ents.append((f'ir{k}', spf, spf))
        else:
            for hh in range(2):
                ents.append((f'ft{k}_{hh}', 128, spf))
                ents.append((f'ir{k}_{hh}', 128, spf))
        ents.append((f'pb{k}', 128, 512))
        ents.append((f'ab{k}', 128, 512))
    off = {}
    o = 0
    for name, r, cdim in ents:
        off[name] = (r, o, cdim)
        o += cdim
    return off, o


MEGA_OFF, MEGA_COLS = _mega_layout()


def _band_bases(bs):
    u = bs // 32
    Bc = 512 // u
    gm, g0, gp = _interp_vecs(u)
    Gm, G0, Gp = np.cumsum(gm), np.cumsum(g0), np.cumsum(gp)
    pb = np.zeros((4 * Bc, 512))
    ab = np.zeros((3 * Bc, 512))
    inv = 1.0 / ADJUST[bs]
    for qq in range(Bc):
        cols = slice(qq * u, (qq + 1) * u)
        pb[0 * Bc + qq, cols] = Gm
        pb[1 * Bc + qq, cols] = G0
        pb[2 * Bc + qq, cols] = Gp
        pb[3 * Bc + qq, cols] = 1.0
        ab[0 * Bc + qq, cols] = gm * inv
        ab[1 * Bc + qq, cols] = g0 * inv
        ab[2 * Bc + qq, cols] = gp * inv
    # replicate at each slot offset so lhsT/rhs share base partitions
    shp, spcp = _slot_cfg(4 * Bc)
    sha, spca = _slot_cfg(3 * Bc)
    pbr = np.zeros((128, 512))
    abr = np.zeros((128, 512))
    for s in range(spcp):
        pbr[s * shp:s * shp + 4 * Bc] = pb
    for s in range(spca):
        abr[s * sha:s * sha + 3 * Bc] = ab
    return pbr.astype(np.float32), abr.astype(np.float32)


def _band_fir(bs):
    spf = bs // NNF
    nc_ = spf // 2 + 1
    t = np.arange(spf)
    j_re = np.arange(nc_)
    j_im = np.arange(1, nc_ - 1)
    FT = np.concatenate([np.cos(2 * np.pi * np.outer(t, j_re) / spf),
                         -np.sin(2 * np.pi * np.outer(t, j_im) / spf)], axis=1)
    w = np.full(nc_, 2.0)
    w[0] = 1.0
    w[-1] = 1.0
    IR = np.concatenate([
        (w[:, None] * np.cos(2 * np.pi * np.outer(j_re, t) / spf)) / spf,
        (-2.0 * np.sin(2 * np.pi * np.outer(j_im, t) / spf)) / spf,
    ], axis=0) / ADJUST[bs]
    return FT.astype(np.float32), IR.astype(np.float32)


def _build_shared(inp):
    c = {}
    wl = np.zeros((4, C + 1, C), np.float32)
    for t in range(4):
        wl[t, :C] = inp['up_lin_w'][:, t::4]
        wl[t, C] = inp['up_lin_b'][t::4]
    c['wlin'] = wl.transpose(1, 0, 2).reshape(C + 1, 4 * C)   # (65, 256), block t
    for n in (4, 8, 16):
        U = _build_U(n)
        ub = np.zeros((BL * n, BL * 2 * n), np.float32)
        for b in range(BL):
            ub[b * n:(b + 1) * n, b * 2 * n:(b + 1) * 2 * n] = U
        c[f'ubd{n}'] = ub

    w64 = np.zeros((C, len(W64_ORDER) * C), np.float32)

    def put64(name, m):
        i = W64_IDX[name]
        w64[:, i * C:(i + 1) * C] = m

    for i in range(3):
        for dd in range(3):
            put64(f'up{i}d{dd}', inp['up_conv_w'][i, :, :, dd].T)
    for dd in range(3):
        put64(f'find{dd}', inp['up_final_w'][:, :, dd].T)
    for k in range(6):
        for j in range(4):
            put64(f't{k}{j}', inp['t_w'][k, j].T)
        put64(f'bf{k}', inp['band_final_w'][k].T)
        put64(f'nup{k}', inp['noise_up_w'][k].T)
    put64('ident', np.eye(C))
    c['w64'] = w64

    w32 = np.zeros((C, 12 * N_OSC), np.float32)
    for k in range(6):
        w32[:, W3264_IDX[f'amp{k}'] * N_OSC:(W3264_IDX[f'amp{k}'] + 1) * N_OSC] = inp['osc_amp_w'][k].T
        w32[:, W3264_IDX[f'frq{k}'] * N_OSC:(W3264_IDX[f'frq{k}'] + 1) * N_OSC] = inp['osc_freq_w'][k].T
    c['w3264'] = w32

    b64 = np.zeros((C, len(B64_ORDER)), np.float32)
    for i in range(3):
        b64[:, B64_IDX[f'up{i}']] = inp['up_conv_b'][i]
    b64[:, B64_IDX['fin']] = inp['up_final_b']
    for k in range(6):
        for j in range(4):
            b64[:, B64_IDX[f't{k}{j}']] = inp['t_b'][k, j]
        b64[:, B64_IDX[f'bf{k}']] = inp['band_final_b'][k]
        b64[:, B64_IDX[f'nup{k}']] = inp['noise_up_b'][k]
    c['bias64'] = b64

    b32 = np.zeros((N_OSC, 12), np.float32)
    for k in range(6):
        b32[:, W3264_IDX[f'amp{k}']] = inp['osc_amp_b'][k]
        b32[:, W3264_IDX[f'frq{k}']] = inp['osc_freq_b'][k]
    c['bias32'] = b32

    for k, bs in enumerate(BAND_SIZES):
        nc_ = bs // NNF // 2 + 1
        w = np.zeros((C + 1, nc_), np.float32)
        w[:C] = inp[f'noise_coeff_w_{k}'].T
        w[C] = inp[f'noise_coeff_b_{k}']
        if k == 0:
            w[:, 1:] = 0.0
        c[f'wc{k}'] = w
        FT, IR = _band_fir(bs)
        c[f'ft{k}'] = FT
        c[f'ir{k}'] = IR
        pb, ab = _band_bases(bs)
        c[f'pb{k}'] = pb
        c[f'ab{k}'] = ab

    c['negI'] = (-np.eye(128)).astype(np.float32)
    c['ident128'] = np.eye(128, dtype=np.float32)
    sel = np.zeros((128, 256), np.float32)
    for b in range(BL):
        sel[b * N_OSC:(b + 1) * N_OSC, 128 + b] = 1.0
    c['selstrip'] = sel
    # split band-5 DFT mats into row halves
    c['ft5_0'], c['ft5_1'] = c['ft5'][0:128], c['ft5'][128:256]
    c['ir5_0'], c['ir5_1'] = c['ir5'][0:128], c['ir5'][128:256]
    mega = np.zeros((128, MEGA_COLS), np.float32)
    for name, (r, o, cd) in MEGA_OFF.items():
        mega[0:r, o:o + cd] = c[name]
    return {'mega': mega}


# ---------------------------------------------------------------- bass build
def _build_nc():
    nc = bacc.Bacc('TRN2', num_devices=NCORE)
    AF = mybir.ActivationFunctionType

    d = {}
    d['xT'] = nc.dram_tensor("xT", [C + 1, BL], F32, kind="ExternalInput")
    d['mega'] = nc.dram_tensor("mega", [128, MEGA_COLS], F32, kind="ExternalInput")
    for k, bs in enumerate(BAND_SIZES):
        spf = bs // NNF
        d[f'noise{k}'] = nc.dram_tensor(f"noise{k}", [FR, spf], F32, kind="ExternalInput")
    out_d = nc.dram_tensor("out", [BL, TOTAL], F32, kind="ExternalOutput")

    with TileContext(nc) as tc:
        with tc.tile_pool(name="const", bufs=1) as cp, \
             tc.tile_pool(name="work", bufs=1) as wp, \
             tc.tile_pool(name="hot", bufs=4) as hot, \
             tc.tile_pool(name="dram", bufs=1, space="DRAM") as dp, \
             tc.tile_pool(name="pp", bufs=3, space="PSUM") as pp, \
             tc.tile_pool(name="pa", bufs=2, space="PSUM") as pa, \
             tc.tile_pool(name="ph", bufs=3, space="PSUM") as ph:

            # ---------------- const loads: one mega tile + bf16 casts
            mega = cp.tile([128, MEGA_COLS], F32, tag="mega")
            _nsplit = 4
            _cut = [MEGA_COLS * i // _nsplit for i in range(_nsplit + 1)]
            for _i in range(_nsplit):
                nc.gpsimd.dma_start(out=mega[:, _cut[_i]:_cut[_i + 1]],
                                    in_=d['mega'][:, _cut[_i]:_cut[_i + 1]])

            ct = {}
            for name, (r, o, cd) in MEGA_OFF.items():
                ct[name] = mega[0:r, o:o + cd]
            # bf16 casts for the cheap matmul paths
            for name in ('negI', 'selstrip'):
                r, o, cd = MEGA_OFF[name]
                t = cp.tile([r, cd], BF16, tag=f"bf_{name}")
                nc.gpsimd.dma_start(out=t, in_=d['mega'][0:r, o:o + cd])
                ct[f'{name}_bf'] = t
            for k in range(6):
                r, o, cd = MEGA_OFF[f'ab{k}']
                t = cp.tile([r, cd], BF16, tag=f"bf_ab{k}")
                nc.gpsimd.dma_start(out=t, in_=d['mega'][0:r, o:o + cd])
                ct[f'ab{k}_bf'] = t

            def w64s(name):
                i = W64_IDX[name]
                return ct['w64'][:, i * C:(i + 1) * C]

            def w32s(name):
                i = W3264_IDX[name]
                return ct['w3264'][:, i * N_OSC:(i + 1) * N_OSC]

            def b64s(name):
                return ct['bias64'][:, B64_IDX[name]:B64_IDX[name] + 1]

            def b32s(name):
                i = W3264_IDX[name]
                return ct['bias32'][:, i:i + 1]

            ident64 = w64s('ident')

            xT = cp.tile([C + 1, BL], F32, tag="xT")
            nc.sync.dma_start(out=xT, in_=d['xT'][:, :])

            # ---------------- frontend
            h = wp.tile([C, 16], F32, tag="h0")
            for t in range(4):
                pt = pp.tile([C, BL], F32, tag="pp")
                nc.tensor.matmul(out=pt, lhsT=ct['wlin'][:, t * C:(t + 1) * C], rhs=xT,
                                 start=True, stop=True)
                nc.vector.tensor_copy(out=h.rearrange("c (b t) -> c b t", t=4)[:, :, t], in_=pt)

            for i, n in enumerate((4, 8, 16)):
                pt1 = pp.tile([BL * n, C], F32, tag="pp")
                nc.tensor.transpose(out=pt1, in_=h, identity=ident64)
                t1 = wp.tile([BL * n, C], F32, tag=f"fe_t1_{i}")
                nc.vector.tensor_copy(out=t1, in_=pt1)
                pt2 = pp.tile([BL * 2 * n, C], F32, tag="pp")
                nc.tensor.matmul(out=pt2, lhsT=ct[f'ubd{n}'], rhs=t1, start=True, stop=True)
                t2 = wp.tile([BL * 2 * n, C], F32, tag=f"fe_t2_{i}")
                nc.vector.tensor_copy(out=t2, in_=pt2)
                pt3 = pp.tile([C, BL * 2 * n], F32, tag="pp")
                nc.tensor.transpose(out=pt3, in_=t2,
                                    identity=ct['ident128'][0:BL * 2 * n, 0:BL * 2 * n])
                hu = wp.tile([C, BL * 2 * n], F32, tag=f"fe_hu_{i}")
                nc.vector.tensor_copy(out=hu, in_=pt3)
                m = 2 * n
                hu3 = hu.rearrange("c (b t) -> c b t", b=BL)
                pc = pp.tile([C, BL, m], F32, tag="pp")
                nc.tensor.matmul(out=pc[:, :, :], lhsT=w64s(f'up{i}d1'), rhs=hu3[:, :, :],
                                 start=True, stop=False)
                nc.tensor.matmul(out=pc[:, :, 1:m], lhsT=w64s(f'up{i}d0'), rhs=hu3[:, :, 0:m - 1],
                                 start=False, stop=False)
                nc.tensor.matmul(out=pc[:, :, 0:m - 1], lhsT=w64s(f'up{i}d2'), rhs=hu3[:, :, 1:m],
                                 start=False, stop=True)
                h = wp.tile([C, BL * m], F32, tag=f"fe_h_{i}")
                nc.scalar.activation(out=h.rearrange("c (b t) -> c b t", b=BL), in_=pc,
                                     func=AF.Prelu, bias=b64s(f'up{i}'), scale=1.0, alpha=0.2)

            h3 = h.rearrange("c (b t) -> c b t", b=BL)
            pf = pp.tile([C, BL, 32], F32, tag="pp")
            nc.tensor.matmul(out=pf[:, :, :], lhsT=w64s('find1'), rhs=h3[:, :, :],
                             start=True, stop=False)
            nc.tensor.matmul(out=pf[:, :, 1:32], lhsT=w64s('find0'), rhs=h3[:, :, 0:31],
                             start=False, stop=False)
            nc.tensor.matmul(out=pf[:, :, 0:31], lhsT=w64s('find2'), rhs=h3[:, :, 1:32],
                             start=False, stop=True)
            hfin = cp.tile([C, 128], F32, tag="hfin")
            nc.scalar.activation(out=hfin.rearrange("c (b t) -> c b t", b=BL), in_=pf,
                                 func=AF.Identity, bias=b64s('fin'), scale=1.0)

            zt32 = cp.tile([N_OSC, 32], F32, tag="zt32")
            nc.vector.memset(zt32, 0.0)

            band_lhsT = {}

            # ---------------- per-band: residual stack, grids, noise
            for k, bs in enumerate(BAND_SIZES):
                u = bs // 32
                Bc = 512 // u
                nch = bs // 512
                spf = bs // NNF
                nc_ = spf // 2 + 1
                lf = 0.05 if bs == 512 else 0.01

                z = hfin
                for j in range(4):
                    pz = pp.tile([C, 128], F32, tag="pp")
                    nc.tensor.matmul(out=pz, lhsT=w64s(f't{k}{j}'), rhs=z, start=True, stop=False)
                    nc.tensor.matmul(out=pz, lhsT=ident64, rhs=z, start=False, stop=True)
                    z = wp.tile([C, 128], F32, tag=f"z_{j % 2}")
                    nc.scalar.activation(out=z, in_=pz, func=AF.Prelu,
                                         bias=b64s(f't{k}{j}'), scale=1.0, alpha=0.2)
                pz = pp.tile([C, 128], F32, tag="pp")
                nc.tensor.matmul(out=pz, lhsT=w64s(f'bf{k}'), rhs=z, start=True, stop=True)
                zf = wp.tile([C, 128], F32, tag="zf")
                nc.scalar.activation(out=zf, in_=pz, func=AF.Identity, bias=b64s(f'bf{k}'),
                                     scale=1.0)

                pg = pa.tile([N_OSC, 128], F32, tag="pa")
                nc.tensor.matmul(out=pg, lhsT=w32s(f'amp{k}'), rhs=zf, start=True, stop=True)
                ampg = wp.tile([N_OSC, 128], F32, tag="ampg")
                nc.scalar.activation(out=ampg, in_=pg, func=AF.Abs, bias=b32s(f'amp{k}'), scale=1.0)
                pg2 = pa.tile([N_OSC, 128], F32, tag="pa")
                nc.tensor.matmul(out=pg2, lhsT=w32s(f'frq{k}'), rhs=zf, start=True, stop=True)
                sigg = wp.tile([N_OSC, 128], F32, tag="sigg")
                nc.scalar.activation(out=sigg, in_=pg2, func=AF.Sigmoid, bias=b32s(f'frq{k}'),
                                     scale=1.0)

                def shifted(src, tagn):
                    pr = wp.tile([N_OSC, 128], F32, tag=f"{tagn}pr")
                    nx = wp.tile([N_OSC, 128], F32, tag=f"{tagn}nx")
                    s3 = src.rearrange("o (b q) -> o b q", b=BL)
                    p3 = pr.rearrange("o (b q) -> o b q", b=BL)
                    n3 = nx.rearrange("o (b q) -> o b q", b=BL)
                    nc.vector.tensor_copy(out=p3[:, :, 1:32], in_=s3[:, :, 0:31])
                    nc.vector.tensor_copy(out=p3[:, :, 0:1], in_=s3[:, :, 0:1])
                    nc.vector.tensor_copy(out=n3[:, :, 0:31], in_=s3[:, :, 1:32])
                    nc.vector.tensor_copy(out=n3[:, :, 31:32], in_=s3[:, :, 31:32])
                    return pr, nx

                incg = wp.tile([N_OSC, 128], F32, tag="incg")
                nc.vector.tensor_scalar(out=incg, in0=sigg, scalar1=float((1.0 - lf) / 2.0),
                                        scalar2=float(lf / 2.0), op0=ALU.mult, op1=ALU.add)
                ipr, inx = shifted(incg, "i")
                apr, anx = shifted(ampg, "a")

                t1 = wp.tile([N_OSC, 128], F32, tag="sg1")
                nc.vector.tensor_tensor(out=t1, in0=ipr, in1=inx, op=ALU.add)
                t2 = wp.tile([N_OSC, 128], F32, tag="sg2")
                nc.vector.tensor_scalar(out=t2, in0=t1, scalar1=float(u / 8.0), scalar2=None,
                                        op0=ALU.mult)
                t3 = wp.tile([N_OSC, 128], F32, tag="sg3")
                nc.vector.tensor_scalar(out=t3, in0=incg, scalar1=float(3.0 * u / 4.0),
                                        scalar2=None, op0=ALU.mult)
                S = wp.tile([N_OSC, 128], F32, tag="sgS")
                nc.vector.tensor_tensor(out=S, in0=t2, in1=t3, op=ALU.add)
                Sn = wp.tile([N_OSC, 128], F32, tag="sgSn")
                nc.vector.tensor_scalar(out=Sn, in0=S, scalar1=MAGIC, scalar2=MAGIC,
                                        op0=ALU.add, op1=ALU.subtract)
                Sr = wp.tile([N_OSC, 128], F32, tag="sgSr")
                nc.vector.tensor_tensor(out=Sr, in0=S, in1=Sn, op=ALU.subtract)
                Pt = wp.tile([N_OSC, 128], F32, tag="sgPt")
                for b in range(BL):
                    nc.vector.tensor_tensor_scan(out=Pt[:, 32 * b:32 * (b + 1)],
                                                 data0=Sr[:, 32 * b:32 * (b + 1)], data1=zt32,
                                                 initial=0.0, op0=ALU.add, op1=ALU.add)
                Ce = wp.tile([N_OSC, 128], F32, tag="sgCe")
                Ce3 = Ce.rearrange("o (b q) -> o b q", b=BL)
                nc.vector.tensor_copy(out=Ce3[:, :, 1:32],
                                      in_=Pt.rearrange("o (b q) -> o b q", b=BL)[:, :, 0:31])
                nc.vector.memset(Ce3[:, :, 0:1], 0.0)
                Cn = wp.tile([N_OSC, 128], F32, tag="sgCn")
                nc.vector.tensor_scalar(out=Cn, in0=Ce, scalar1=MAGIC, scalar2=MAGIC,
                                        op0=ALU.add, op1=ALU.subtract)
                Cf = wp.tile([N_OSC, 128], F32, tag="sgCf")
                nc.vector.tensor_tensor(out=Cf, in0=Ce, in1=Cn, op=ALU.subtract)

                Tall = wp.tile([N_OSC, 7 * 128], F32, tag="Tall")
                for g, grid in enumerate((ipr, incg, inx, Cf, apr, ampg, anx)):
                    nc.vector.transpose(out=Tall[:, 128 * g:128 * (g + 1)], in_=grid)
                scr = dp.tile([N_OSC, 7 * 128], F32, tag=f"scr_{k}")
                nc.gpsimd.dma_start(out=scr, in_=Tall)

                shp, spcp = _slot_cfg(4 * Bc)
                sha, spca = _slot_cfg(3 * Bc)
                ncolp = -(-nch // spcp)
                ncola = -(-nch // spca)
                lp = cp.tile([128, 128 * ncolp], F32, tag=f"lp_{k}")
                la = cp.tile([128, 128 * ncola], BF16, tag=f"la_{k}")
                dmae = [nc.gpsimd, nc.sync]
                di = 0
                for g in range(4):
                    for s in range(min(spcp, nch)):
                        cnt = len(range(s, nch, spcp))
                        dmae[di % 2].dma_start(
                            out=lp[s * shp + g * Bc:s * shp + (g + 1) * Bc, :]
                            .rearrange("q (c m) -> q c m", m=128)[:, 0:cnt, :],
                            in_=scr[:, 128 * g:128 * (g + 1)]
                            .rearrange("(c q) m -> q c m", q=Bc)[:, s::spcp, :])
                        di += 1
                for g in range(3):
                    for s in range(min(spca, nch)):
                        cnt = len(range(s, nch, spca))
                        nc.gpsimd.dma_start(
                            out=la[s * sha + g * Bc:s * sha + (g + 1) * Bc, :]
                            .rearrange("q (c m) -> q c m", m=128)[:, 0:cnt, :],
                            in_=scr[:, 128 * (4 + g):128 * (5 + g)]
                            .rearrange("(c q) m -> q c m", q=Bc)[:, s::spca, :])
                        di += 1
                band_lhsT[k] = (lp, la)

                # ---------------- noise branch
                zf3 = zf.rearrange("c (b t) -> c b t", b=BL)
                zrep = zf3.unsqueeze(-1).broadcast_to([C, BL, 32, 2])
                pn = pp.tile([C, FR], F32, tag="pp")
                nc.tensor.matmul(out=pn, lhsT=w64s(f'nup{k}'), rhs=zrep, start=True, stop=True)
                naug = wp.tile([C + 1, FR], F32, tag="naug")
                nc.scalar.activation(out=naug[0:C, :], in_=pn, func=AF.Prelu,
                                     bias=b64s(f'nup{k}'), scale=1.0, alpha=0.2)
                nc.vector.memset(naug[C:C + 1, :], 1.0)

                nb0 = wp.tile([128, spf], F32, tag="nb0")
                nb1 = wp.tile([128, spf], F32, tag="nb1")
                nc.sync.dma_start(out=nb0, in_=d[f'noise{k}'][0:128, :])
                nc.sync.dma_start(out=nb1, in_=d[f'noise{k}'][128:256, :])
                if spf <= 128:
                    nT = wp.tile([spf, FR], F32, tag="nT")
                    for hh, nb in enumerate((nb0, nb1)):
                        ptr = pp.tile([spf, 128], F32, tag="pp")
                        nc.tensor.transpose(out=ptr, in_=nb, identity=ct['ident128'])
                        nc.vector.tensor_copy(out=nT[:, 128 * hh:128 * (hh + 1)], in_=ptr)
                    nTs = [nT]
                else:
                    nT0 = wp.tile([128, FR], F32, tag="nT5_0")
                    nT1 = wp.tile([128, FR], F32, tag="nT5_1")
                    for hh, nb in enumerate((nb0, nb1)):
                        for half, dst in ((0, nT0), (1, nT1)):
                            ptr = pp.tile([128, 128], F32, tag="pp")
                            nc.tensor.transpose(out=ptr, in_=nb[:, 128 * half:128 * (half + 1)],
                                                identity=ct['ident128'])
                            nc.vector.tensor_copy(out=dst[:, 128 * hh:128 * (hh + 1)], in_=ptr)
                    nTs = [nT0, nT1]

                if nc_ <= 128:
                    pcA = pa.tile([nc_, FR], F32, tag="pa")
                    nc.tensor.matmul(out=pcA, lhsT=ct[f'wc{k}'], rhs=naug, start=True, stop=True)
                    pcB = None
                else:
                    pcA = pa.tile([128, FR], F32, tag="pa")
                    nc.tensor.matmul(out=pcA, lhsT=ct[f'wc{k}'][:, 0:128], rhs=naug,
                                     start=True, stop=True)
                    pcB = pp.tile([1, FR], F32, tag="pp")
                    nc.tensor.matmul(out=pcB, lhsT=ct[f'wc{k}'][:, 128:nc_], rhs=naug,
                                     start=True, stop=True)

                if spf <= 128:
                    psp = pp.tile([spf, FR], F32, tag="pp")
                    nc.tensor.matmul(out=psp, lhsT=ct[f'ft{k}'], rhs=nTs[0], start=True, stop=True)
                    chat = wp.tile([spf, FR], F32, tag="chat")
                    nc.vector.tensor_copy(out=chat[0:nc_, :], in_=pcA)
                    if spf > nc_:
                        nc.sync.dma_start(out=chat[nc_:spf, :], in_=chat[1:nc_ - 1, :])
                    sA = wp.tile([spf, FR], F32, tag="sA")
                    nc.vector.tensor_tensor(out=sA, in0=chat, in1=psp, op=ALU.mult)
                    sAs = [sA]
                else:
                    sAs = []
                    for half in range(2):
                        psp = pp.tile([128, FR], F32, tag="pp")
                        nc.tensor.matmul(out=psp, lhsT=ct['ft5_0'][:, 128 * half:128 * (half + 1)],
                                         rhs=nTs[0], start=True, stop=False)
                        nc.tensor.matmul(out=psp, lhsT=ct['ft5_1'][:, 128 * half:128 * (half + 1)],
                                         rhs=nTs[1], start=False, stop=True)
                        ch = wp.tile([128, FR], F32, tag=f"chat5_{half}")
                        if half == 0:
                            nc.vector.tensor_copy(out=ch, in_=pcA)
                        else:
                            # rows [c_128, c_1..c_127]: full copy then overwrite row 0
                            nc.vector.tensor_copy(out=ch, in_=pcA)
                            nc.vector.tensor_copy(out=ch[0:1, :], in_=pcB)
                        sA = wp.tile([128, FR], F32, tag=f"sA5_{half}")
                        nc.vector.tensor_tensor(out=sA, in0=ch, in1=psp, op=ALU.mult)
                        sAs.append(sA)

                for fg in range(2):
                    pnz = ph.tile([128, spf], F32, tag="ph")
                    if spf <= 128:
                        nc.tensor.matmul(out=pnz, lhsT=sAs[0][:, 128 * fg:128 * (fg + 1)],
                                         rhs=ct[f'ir{k}'], start=True, stop=True)
                    else:
                        nc.tensor.matmul(out=pnz, lhsT=sAs[0][:, 128 * fg:128 * (fg + 1)],
                                         rhs=ct['ir5_0'], start=True, stop=False)
                        nc.tensor.matmul(out=pnz, lhsT=sAs[1][:, 128 * fg:128 * (fg + 1)],
                                         rhs=ct['ir5_1'], start=False, stop=True)
                    nzs = wp.tile([128, spf], F32, tag="nzs")
                    nc.scalar.copy(out=nzs, in_=pnz)
                    for j in range(2):
                        b_ = 2 * fg + j
                        nc.sync.dma_start(
                            out=out_d[b_:b_ + 1, NZ_OFF[k]:NZ_OFF[k] + bs]
                            .rearrange("o (f t) -> (o f) t", t=spf),
                            in_=nzs[NNF * j:NNF * (j + 1), :])

            # ---------------- harmonic chunks
            pending = []
            osc_q = []

            def _emit_osc():
                while osc_q:
                    osc_q.pop(0)()

            def _flush_harm():
                for (hpt_, gs_, gstart_, k_) in pending:
                    hsb = wp.tile([4 * gs_, 512], F32, tag="hsb")
                    nc.scalar.copy(out=hsb, in_=hpt_[0:4 * gs_, :])
                    for b_ in range(BL):
                        nc.sync.dma_start(
                            out=out_d[b_:b_ + 1,
                                      HARM_OFF[k_] + 512 * gstart_:
                                      HARM_OFF[k_] + 512 * (gstart_ + gs_)]
                            .rearrange("o (cc t) -> (o cc) t", t=512),
                            in_=hsb[b_:4 * gs_:4, :])
                pending.clear()

            for k, bs in enumerate(BAND_SIZES):
                Bc = 512 // (bs // 32)
                nch = bs // 512
                lp, la = band_lhsT[k]
                pbt = ct[f'pb{k}']
                abt = ct[f'ab{k}_bf']
                shp, spcp = _slot_cfg(4 * Bc)
                sha, spca = _slot_cfg(3 * Bc)
                for gstart in range(0, nch, 8):
                    gs = min(8, nch - gstart)
                    hpt = ph.tile([128, 512], F32, tag="ph")
                    for cc in range(gs):
                        if cc == 2:
                            _flush_harm()
                        c_ = gstart + cc
                        sp_, colp = c_ % spcp, c_ // spcp
                        sa_, cola = c_ % spca, c_ // spca
                        ppt = pp.tile([128, 512], F32, tag="pp")
                        nc.tensor.matmul(out=ppt,
                                         lhsT=lp[sp_ * shp:sp_ * shp + 4 * Bc,
                                                 128 * colp:128 * (colp + 1)],
                                         rhs=pbt[sp_ * shp:sp_ * shp + 4 * Bc, :],
                                         start=True, stop=False)
                        ntile = hot.tile([128, 512], BF16, tag="ntile")
                        nc.vector.tensor_scalar(out=ntile, in0=ppt, scalar1=MAGIC, scalar2=MAGIC,
                                                op0=ALU.add, op1=ALU.subtract)
                        nc.tensor.matmul(out=ppt, lhsT=ct['negI_bf'], rhs=ntile,
                                         start=False, stop=True)
                        s = hot.tile([128, 512], F32, tag="sin_t")
                        nc.scalar.activation(out=s, in_=ppt, func=AF.Sin, scale=TWO_PI)
                        pat = pa.tile([128, 512], F32, tag="pa")
                        nc.tensor.matmul(out=pat,
                                         lhsT=la[sa_ * sha:sa_ * sha + 3 * Bc,
                                                 128 * cola:128 * (cola + 1)],
                                         rhs=abt[sa_ * sha:sa_ * sha + 3 * Bc, :],
                                         start=True, stop=True)
                        prod = hot.tile([128, 512], BF16, tag="prod_t")
                        nc.vector.tensor_tensor(out=prod, in0=s, in1=pat, op=ALU.mult)

                        def _mk(hpt_=hpt, cc_=cc, prod_=prod, st=(cc == 0), sp=(cc == gs - 1)):
                            def _f():
                                nc.tensor.matmul(
                                    out=hpt_,
                                    lhsT=ct['selstrip_bf'][:, 128 - 4 * cc_:256 - 4 * cc_],
                                    rhs=prod_, start=st, stop=sp, skip_group_check=True)
                            return _f
                        prev_osc = osc_q.pop(0) if osc_q else None
                        osc_q.append(_mk())
                        if prev_osc is not None:
                            prev_osc()
                    pending.append((hpt, gs, gstart, k))
                    if gs < 3:
                        _emit_osc()
                        _flush_harm()
            _emit_osc()
            _flush_harm()

    nc.finalize()
    return nc


def _prep_inputs(inputs):
    inp = {k: np.asarray(v, np.float32) for k, v in inputs.items()}
    shared = _build_shared(inp)
    in_maps = []
    for core in range(NCORE):
        m = dict(shared)
        sl = slice(core * BL, (core + 1) * BL)
        m['xT'] = np.concatenate([inp['x'][sl].T, np.ones((1, BL), np.float32)], axis=0)
        for k, bs in enumerate(BAND_SIZES):
            spf = bs // NNF
            m[f'noise{k}'] = np.ascontiguousarray(inp[f'noise_{k}'][sl].reshape(FR, spf),
                                                  dtype=np.float32)
        in_maps.append(m)
    return in_maps


def kernel(**inputs):
    if 'nc' not in _nc_cache:
        _nc_cache['nc'] = _build_nc()
    nc = _nc_cache['nc']
    in_maps = _prep_inputs(inputs)
    res = run_bass_kernel_spmd(nc, in_maps, list(range(NCORE)))
    out = np.concatenate([res.results[i]["out"] for i in range(NCORE)], axis=0)
    return out.astype(np.float32)


if __name__ == "__main__":
    import reference
    inp = reference.setup_inputs()
    out = kernel(**{k: np.asarray(v) for k, v in inp.items()})
    print("out", out.shape, out.dtype)

